# revision 5
# baseline (speedup 1.0000x reference)
"""C3D-style circulant-block 3D CNN forward pass on 8 Trainium2 NeuronCores.

Sharding: data-parallel over batch (8 samples -> 8 cores). Training-mode
BatchNorm batch statistics are combined across cores with a tiny per-layer
f32 AllReduce of (mean, E[x^2]) per channel.

Device kernel per core (per sample):
  conv1 via host-side im2col (K=81 = ci*kd*kh*kw) -> single matmul stream
  conv2..conv5b as shift-and-accumulate implicit GEMM: input channels on
  partitions (K-chunks of 128), 27 taps accumulated in PSUM, strided
  interior access patterns on padded SBUF buffers. conv2 packs (ci, 2 kd
  planes) into K=128 + a K=64 remainder.
  Per conv tile: ACT copies PSUM->bf16, DVE bn_stats accumulates BN stats,
  DVE tensor_max performs maxpool (pool applied to raw values before the
  BN affine; valid because the BN scale g*rsqrt(var+eps) is positive).
  After each conv: bn_aggr -> AllReduce(mean, E2) -> scale/shift -> fused
  BN+ReLU via one ACT op writing the next layer's padded input.
  Tail: special-padded pool5 -> global mean (folded /16 into FC weights)
  -> FC matmul -> logits.
"""

import numpy as np
import ml_dtypes

import concourse.bass as bass
import concourse.mybir as mybir
import concourse.tile as tile
from concourse import bacc
from concourse.bass_utils import run_bass_kernel_spmd

F32 = mybir.dt.float32
BF16 = mybir.dt.bfloat16
NPBF16 = ml_dtypes.bfloat16
RELU = mybir.ActivationFunctionType.Relu
COPY = mybir.ActivationFunctionType.Copy
SQRT = mybir.ActivationFunctionType.Sqrt
IDENT = mybir.ActivationFunctionType.Identity
ADD = mybir.AluOpType.add
EPS = 1e-5
N_CORES = 8

# name, Cin, Cout, D, H, W, R(rows/tile), zpair, pooled
GEN_LAYERS = [
    ("3a", 128, 256, 8, 28, 28, 14, False, False),
    ("3b", 256, 256, 8, 28, 28, 14, False, True),
    ("4a", 256, 512, 4, 14, 14, 14, True, False),
    ("4b", 512, 512, 4, 14, 14, 14, True, True),
    ("5a", 512, 512, 2, 7, 7, 7, True, False),
    ("5b", 512, 512, 2, 7, 7, 7, True, None),  # None -> raw stage (special pool)
]

TAPS = [(kd, kh, kw) for kd in range(3) for kh in range(3) for kw in range(3)]


def circ_expand_np(c):
    c = np.asarray(c, np.float32)
    P, Q, b = c.shape[0], c.shape[1], c.shape[2]
    r = np.arange(b)
    idx = (r[:, None] - r[None, :]) % b
    w = c[:, :, idx]  # (P, Q, b, b, k, k, k)
    w = np.transpose(w, (0, 2, 1, 3, 4, 5, 6))
    return w.reshape(P * b, Q * b, *c.shape[3:])


def pack_w_generic(wd, Kch, Mch):
    # wd (Co, Ci, 3,3,3) -> [Mch, 128ci, Kch, 27, 128co] bf16
    Co, Ci = wd.shape[0], wd.shape[1]
    wt = wd.transpose(1, 2, 3, 4, 0)  # (Ci, kd,kh,kw, Co)
    wt = wt.reshape(Kch, 128, 27, Mch, 128)
    wt = wt.transpose(3, 1, 0, 2, 4)  # (m, ci, c, t, co)
    return np.ascontiguousarray(wt, dtype=NPBF16)


def host_prep(inputs):
    g = {k: np.asarray(v, np.float32) for k, v in inputs.items()}
    shared = {}
    # conv1
    w1 = g["conv1_w"]  # (64, 3, 3,3,3)
    shared["w1"] = np.ascontiguousarray(
        w1.transpose(1, 2, 3, 4, 0).reshape(81, 64), dtype=NPBF16)
    # conv2 (kd-packed)
    w2 = circ_expand_np(g["c2"])  # (128, 64, 3,3,3)
    w2t = w2.transpose(2, 1, 3, 4, 0)  # (kd, ci, kh, kw, co)
    shared["w2a"] = np.ascontiguousarray(
        w2t[0:2].reshape(128, 9, 128), dtype=NPBF16)
    shared["w2b"] = np.ascontiguousarray(
        w2t[2].reshape(64, 9, 128), dtype=NPBF16)
    # generic layers
    for (name, Cin, Cout, *_rest) in GEN_LAYERS:
        wd = circ_expand_np(g[f"c{name}"])
        shared[f"w{name}"] = pack_w_generic(wd, Cin // 128, Cout // 128)
    # bn params
    def pk(v, parts):
        v = np.asarray(v, np.float32)
        mch = v.size // parts
        return np.ascontiguousarray(v.reshape(mch, parts).T)
    shared["gn1"] = pk(g["g1"], 64)
    shared["bn1"] = pk(g["b1"], 64)
    for name, c in [("2", 128), ("3a", 256), ("3b", 256), ("4a", 512),
                    ("4b", 512), ("5a", 512), ("5b", 512)]:
        shared[f"gn{name}"] = pk(g[f"g{name}"], 128)
        shared[f"bn{name}"] = pk(g[f"b{name}"], 128)
        assert np.all(g[f"g{name}"] >= 0), "pool/BN commute needs g >= 0"
    assert np.all(g["g1"] >= 0)
    # fc (fold /16 global-mean into weights)
    fcw = (g["fc_w"].T / 16.0)  # (512, 101)
    shared["fcw"] = np.ascontiguousarray(
        fcw.reshape(4, 128, 101).transpose(1, 0, 2), dtype=NPBF16)
    shared["fcb"] = np.ascontiguousarray(g["fc_b"].reshape(101, 1))
    # per-core conv1 im2col
    x = g["x"]  # (8, 3, 16, 112, 112)
    x1_list = []
    for i in range(x.shape[0]):
        xp = np.zeros((3, 18, 114, 114), np.float32)
        xp[:, 1:17, 1:113, 1:113] = x[i]
        sw = np.lib.stride_tricks.sliding_window_view(xp, (3, 3, 3), axis=(1, 2, 3))
        b1 = sw.transpose(0, 4, 5, 6, 1, 2, 3).reshape(81, 16, 12544)
        x1_list.append(np.ascontiguousarray(b1, dtype=NPBF16))
    return shared, x1_list


def build_bass(n_cores, fake_cc=False):
    nc = bacc.Bacc("TRN2", target_bir_lowering=False, debug=False,
                   num_devices=n_cores)
    rg = [list(range(n_cores))]

    din = {}
    din["x1"] = nc.dram_tensor("x1", [81, 16, 12544], BF16, kind="ExternalInput")
    din["w1"] = nc.dram_tensor("w1", [81, 64], BF16, kind="ExternalInput")
    din["w2a"] = nc.dram_tensor("w2a", [128, 9, 128], BF16, kind="ExternalInput")
    din["w2b"] = nc.dram_tensor("w2b", [64, 9, 128], BF16, kind="ExternalInput")
    for (name, Cin, Cout, *_r) in GEN_LAYERS:
        din[f"w{name}"] = nc.dram_tensor(
            f"w{name}", [Cout // 128, 128, Cin // 128, 27, 128], BF16,
            kind="ExternalInput")
    din["gn1"] = nc.dram_tensor("gn1", [64, 1], F32, kind="ExternalInput")
    din["bn1"] = nc.dram_tensor("bn1", [64, 1], F32, kind="ExternalInput")
    for name, c in [("2", 128), ("3a", 256), ("3b", 256), ("4a", 512),
                    ("4b", 512), ("5a", 512), ("5b", 512)]:
        mch = c // 128
        din[f"gn{name}"] = nc.dram_tensor(f"gn{name}", [128, mch], F32,
                                          kind="ExternalInput")
        din[f"bn{name}"] = nc.dram_tensor(f"bn{name}", [128, mch], F32,
                                          kind="ExternalInput")
    din["fcw"] = nc.dram_tensor("fcw", [128, 4, 101], BF16, kind="ExternalInput")
    din["fcb"] = nc.dram_tensor("fcb", [101, 1], F32, kind="ExternalInput")
    logits = nc.dram_tensor("logits", [101, 1], F32, kind="ExternalOutput")
    stats_out = {}
    for name, parts, mch in [("1", 64, 1), ("2", 128, 1), ("3a", 128, 2),
                             ("3b", 128, 2), ("4a", 128, 4), ("4b", 128, 4),
                             ("5a", 128, 4), ("5b", 128, 4)]:
        stats_out[name] = nc.dram_tensor(f"st{name}", [parts, mch * 2], F32,
                                         kind="ExternalOutput")

    with tile.TileContext(nc) as tc:
        build_graph(tc, din, logits, stats_out, rg, fake_cc)
    nc.compile()
    return nc


def build_graph(tc, din, logits, stats_out, rg, fake_cc=False):
    nc = tc.nc
    import contextlib
    ctx = contextlib.ExitStack()
    with ctx:
        singles = ctx.enter_context(tc.tile_pool(name="singles", bufs=1))
        small = ctx.enter_context(tc.tile_pool(name="small", bufs=2))
        statsp = ctx.enter_context(tc.tile_pool(name="statsp", bufs=2))
        psum = ctx.enter_context(tc.tile_pool(name="psum", bufs=6, space="PSUM"))
        psfc = ctx.enter_context(tc.tile_pool(name="psfc", bufs=1, space="PSUM"))
        ybfp = ctx.enter_context(tc.tile_pool(name="ybfp", bufs=4))
        pwp = ctx.enter_context(tc.tile_pool(name="pwp", bufs=4))
        stagep = ctx.enter_context(tc.tile_pool(name="stagep", bufs=1))
        ypoolp = ctx.enter_context(tc.tile_pool(name="ypoolp", bufs=1))
        arena = ctx.enter_context(tc.tile_pool(name="arena", bufs=1))
        dram = ctx.enter_context(tc.tile_pool(name="dram", bufs=1, space="DRAM"))

        eps_t = singles.tile([128, 1], F32, tag="eps")
        nc.vector.memset(eps_t[:], EPS)

        # persistent small params
        params = {}
        for name, parts in [("1", 64), ("2", 128), ("3a", 128), ("3b", 128),
                            ("4a", 128), ("4b", 128), ("5a", 128), ("5b", 128)]:
            mch = din[f"gn{name}"].shape[1]
            gt = singles.tile([parts, mch], F32, tag=f"g{name}")
            bt = singles.tile([parts, mch], F32, tag=f"b{name}")
            nc.sync.dma_start(gt[:], din[f"gn{name}"][:])
            nc.sync.dma_start(bt[:], din[f"bn{name}"][:])
            params[name] = (gt, bt)

        w1_sb = singles.tile([81, 64], BF16, tag="w1")
        nc.sync.dma_start(w1_sb[:], din["w1"][:])
        w2a_sb = singles.tile([128, 9, 128], BF16, tag="w2a")
        nc.sync.dma_start(w2a_sb[:], din["w2a"][:])
        w2b_sb = singles.tile([64, 9, 128], BF16, tag="w2b")
        nc.sync.dma_start(w2b_sb[:], din["w2b"][:])
        fcw_sb = singles.tile([128, 4, 101], BF16, tag="fcw")
        nc.sync.dma_start(fcw_sb[:], din["fcw"][:])
        fcb_sb = singles.tile([101, 1], F32, tag="fcb")
        nc.sync.dma_start(fcb_sb[:], din["fcb"][:])

        y1_dram = dram.tile([64, 16, 3136], BF16, tag="y1d")

        def bn_reduce(name, stats_t, parts, mch):
            """stats_t [parts, mch, T, 6] -> (s, t) [parts, mch] f32."""
            mv = small.tile([parts, mch, 2], F32, tag="mv")
            for m in range(mch):
                nc.vector.bn_aggr(mv[:, m], stats_t[:, m])
            cc = small.tile([parts, mch, 2], F32, tag="cc")
            sq = small.tile([parts, mch], F32, tag="sq")
            nc.vector.tensor_mul(sq[:], mv[:, :, 0], mv[:, :, 0])
            nc.vector.tensor_add(cc[:, :, 1], mv[:, :, 1], sq[:])
            nc.vector.tensor_copy(cc[:, :, 0], mv[:, :, 0])
            ccin = dram.tile([parts, mch * 2], F32, tag=f"ccin{name}")
            ccout = dram.tile([parts, mch * 2], F32, tag=f"ccout{name}",
                              addr_space="Shared")
            nc.sync.dma_start(ccin[:], cc[:].rearrange("p m two -> p (m two)"))
            if fake_cc:
                nc.sync.dma_start(ccout[:], ccin[:])
            else:
                nc.gpsimd.collective_compute(
                    "AllReduce", ADD, replica_groups=rg,
                    ins=[ccin.opt()], outs=[ccout.opt()])
            ar = small.tile([parts, mch, 2], F32, tag="ar")
            nc.sync.dma_start(ar[:].rearrange("p m two -> p (m two)"), ccout[:])
            nc.sync.dma_start(stats_out[name][:],
                              ar[:].rearrange("p m two -> p (m two)"))
            inv_n = 1.0 / len(rg[0])
            mg = small.tile([parts, mch], F32, tag="mg")
            e2 = small.tile([parts, mch], F32, tag="e2")
            nc.vector.tensor_scalar_mul(mg[:], ar[:, :, 0], inv_n)
            nc.vector.tensor_scalar_mul(e2[:], ar[:, :, 1], inv_n)
            sq2 = small.tile([parts, mch], F32, tag="sq2")
            nc.vector.tensor_mul(sq2[:], mg[:], mg[:])
            varg = small.tile([parts, mch], F32, tag="varg")
            nc.vector.tensor_sub(varg[:], e2[:], sq2[:])
            sd = small.tile([parts, mch], F32, tag="sd")
            nc.scalar.activation(sd[:], varg[:], SQRT, bias=eps_t[:parts])
            inv = small.tile([parts, mch], F32, tag="inv")
            nc.vector.reciprocal(inv[:], sd[:])
            gt, bt = params[name]
            s_t = small.tile([parts, mch], F32, tag="s_t")
            nc.vector.tensor_mul(s_t[:], inv[:], gt[:])
            tmn = small.tile([parts, mch], F32, tag="tmn")
            nc.vector.tensor_mul(tmn[:], mg[:], s_t[:])
            t_t = small.tile([parts, mch], F32, tag="t_t")
            nc.vector.tensor_sub(t_t[:], bt[:], tmn[:])
            return s_t, t_t

        def zero_borders(P, mch, Dp, Hp, Wp):
            for c in range(mch):
                nc.vector.memset(P[:, c, 0], 0.0)
                nc.vector.memset(P[:, c, Dp - 1], 0.0)
                nc.vector.memset(P[:, c, 1:Dp - 1, 0, :], 0.0)
                nc.vector.memset(P[:, c, 1:Dp - 1, Hp - 1, :], 0.0)
                nc.vector.memset(P[:, c, 1:Dp - 1, 1:Hp - 1, 0:1], 0.0)
                nc.vector.memset(P[:, c, 1:Dp - 1, 1:Hp - 1, Wp - 1:Wp], 0.0)

        # ---------------- conv1 ----------------
        stats1 = statsp.tile([64, 1, 448, 6], F32, tag="stats")
        with tc.tile_pool(name="x1p", bufs=3) as x1p, \
             tc.tile_pool(name="zplp", bufs=2) as zplp, \
             nc.named_scope("conv1"):
            for z in range(16):
                zplane = zplp.tile([64, 56, 56], BF16, tag="zpl")
                for half in range(2):
                    slab = x1p.tile([81, 6272], BF16, tag="slab")
                    nc.sync.dma_start(
                        slab[:], din["x1"][:, z, half * 6272:(half + 1) * 6272])
                    for t in range(14):
                        ps = psum.tile([64, 448], F32, tag="ps")
                        nc.tensor.matmul(ps[:], w1_sb[:],
                                         slab[:, t * 448:(t + 1) * 448],
                                         start=True, stop=True)
                        ybf = ybfp.tile([64, 448], BF16, tag="ybf")
                        nc.scalar.activation(ybf[:], ps[:], COPY)
                        ti = z * 28 + half * 14 + t
                        nc.vector.bn_stats(stats1[:, 0, ti], ybf[:])
                        v = ybf[:].rearrange("p (a b) -> p a b", a=4)
                        pw = pwp.tile([64, 4, 56], BF16, tag="pw")
                        nc.vector.tensor_max(pw[:], v[:, :, 0::2], v[:, :, 1::2])
                        ro = half * 28 + 2 * t
                        nc.vector.tensor_max(zplane[:, ro:ro + 2, :],
                                             pw[:, 0::2, :], pw[:, 1::2, :])
                nc.sync.dma_start(y1_dram[:, z, :],
                                  zplane[:].rearrange("p a b -> p (a b)"))
        with nc.named_scope("ar1"):
            s1, t1 = bn_reduce("1", stats1, 64, 1)

        # ---------------- conv2 ----------------
        stats2 = statsp.tile([128, 1, 112, 6], F32, tag="stats")
        S2 = stagep.tile([128, 1, 16, 28, 28], BF16, tag="stage")
        with tc.tile_pool(name="plp", bufs=3) as plp, \
             tc.tile_pool(name="b2p", bufs=2) as b2p, \
             tc.tile_pool(name="c2p", bufs=2) as c2p, \
             nc.named_scope("conv2"):

            def build_plane(dst64, pidx):
                if pidx == 0 or pidx == 17:
                    nc.vector.memset(dst64[:], 0.0)
                    return
                pl = plp.tile([64, 3136], BF16, tag="pl")
                nc.sync.dma_start(pl[:], y1_dram[:, pidx - 1, :])
                nc.vector.memset(dst64[:, 0, :], 0.0)
                nc.vector.memset(dst64[:, 57, :], 0.0)
                nc.vector.memset(dst64[:, 1:57, 0:1], 0.0)
                nc.vector.memset(dst64[:, 1:57, 57:58], 0.0)
                nc.scalar.activation(
                    dst64[:, 1:57, 1:57],
                    pl[:].rearrange("p (a b) -> p a b", a=56),
                    RELU, bias=t1[:, 0:1], scale=s1[:, 0:1])

            for z in range(16):
                B2 = b2p.tile([128, 58, 58], BF16, tag="b2")
                build_plane(B2[0:64], z)
                build_plane(B2[64:128], z + 1)
                C2 = c2p.tile([64, 58, 58], BF16, tag="c2")
                build_plane(C2[:], z + 2)
                for ty in range(7):
                    ps = psum.tile([128, 448], F32, tag="ps")
                    for k9 in range(9):
                        kh, kw = k9 // 3, k9 % 3
                        y0 = ty * 8 + kh
                        nc.tensor.matmul(ps[:], w2a_sb[:, k9, :],
                                         B2[:, y0:y0 + 8, kw:kw + 56],
                                         start=(k9 == 0), stop=False)
                    for k9 in range(9):
                        kh, kw = k9 // 3, k9 % 3
                        y0 = ty * 8 + kh
                        nc.tensor.matmul(ps[:], w2b_sb[:, k9, :],
                                         C2[:, y0:y0 + 8, kw:kw + 56],
                                         start=False, stop=(k9 == 8))
                    ybf = ybfp.tile([128, 448], BF16, tag="ybf")
                    nc.vector.tensor_copy(ybf[:], ps[:])
                    nc.vector.bn_stats(stats2[:, 0, z * 7 + ty], ybf[:])
                    v = ybf[:].rearrange("p (a b) -> p a b", a=8)
                    pw = pwp.tile([128, 8, 28], BF16, tag="pw")
                    nc.vector.tensor_max(pw[:], v[:, :, 0::2], v[:, :, 1::2])
                    nc.vector.tensor_max(S2[:, 0, z, 4 * ty:4 * ty + 4, :],
                                         pw[:, 0::2, :], pw[:, 1::2, :])
        with nc.named_scope("ar2"):
            s2, t2 = bn_reduce("2", stats2, 128, 1)
        Y2p = ypoolp.tile([128, 1, 8, 28, 28], BF16, tag="ypool")
        nc.vector.tensor_max(Y2p[:, 0], S2[:, 0, 0::2], S2[:, 0, 1::2])
        P3in = arena.tile([128, 1, 10, 30, 30], BF16, tag="pin")
        zero_borders(P3in, 1, 10, 30, 30)
        nc.scalar.activation(P3in[:, 0, 1:9, 1:29, 1:29], Y2p[:, 0], RELU,
                             bias=t2[:, 0:1], scale=s2[:, 0:1])

        # ---------------- generic conv layers ----------------
        with tc.tile_pool(name="wp", bufs=3) as wp:
            Pin = P3in
            for (name, Cin, Cout, D, H, W, R, zpair, pooled) in GEN_LAYERS:
                Kch, Mch = Cin // 128, Cout // 128
                ntz = D // 2 if zpair else D
                zcnt = 2 if zpair else 1
                ytiles = H // R
                N = zcnt * R * W
                T = ntz * ytiles
                H2, W2, D2 = H // 2, W // 2, D // 2
                stats_t = statsp.tile([128, Mch, T, 6], F32, tag="stats")
                if pooled is False or pooled is None:
                    stage = stagep.tile([128, Mch, D, H, W], BF16, tag="stage")
                else:
                    stage = stagep.tile([128, Mch, D, H2, W2], BF16, tag="stage")
                stage_flat = stage[:].rearrange("p m d h w -> p (m d h w)")
                scope = ctx2 = nc.named_scope(f"conv{name}")
                ctx2.__enter__()
                for m in range(Mch):
                    wm = wp.tile([128, Kch, 27, 128], BF16, tag="w")
                    nc.sync.dma_start(wm[:, :Kch], din[f"w{name}"][m])
                    ti = 0
                    for tz in range(ntz):
                        z0 = 2 * tz if zpair else tz
                        for ty in range(ytiles):
                            y0 = ty * R
                            pst = psum.tile([128, 512], F32, tag="ps",
                                            name="ps")
                            ps = pst[:, :N]
                            nmm = Kch * 27
                            i = 0
                            for c in range(Kch):
                                for (kd, kh, kw) in TAPS:
                                    rhs = Pin[:, c, z0 + kd:z0 + kd + zcnt,
                                              y0 + kh:y0 + kh + R,
                                              kw:kw + W]
                                    nc.tensor.matmul(
                                        ps, wm[:, c, kd * 9 + kh * 3 + kw, :],
                                        rhs, start=(i == 0), stop=(i == nmm - 1))
                                    i += 1
                            if pooled is False or pooled is None:
                                off = (m * D + z0) * H * W + y0 * W
                                dst = stage_flat[:, off:off + N]
                                nc.vector.tensor_copy(dst, ps)
                                nc.vector.bn_stats(stats_t[:, m, ti], dst)
                            else:
                                ybft = ybfp.tile([128, 512], BF16,
                                                 tag="ybf", name="ybf")
                                ybf = ybft[:, :N]
                                nc.vector.tensor_copy(ybf, ps)
                                nc.vector.bn_stats(stats_t[:, m, ti], ybf)
                                v = ybf.rearrange("p (z y x) -> p z y x",
                                                  z=zcnt, y=R)
                                pw = pwp.tile([128, zcnt, R, W2], BF16,
                                              tag="pw2")
                                nc.vector.tensor_max(pw[:], v[:, :, :, 0::2],
                                                     v[:, :, :, 1::2])
                                nc.vector.tensor_max(
                                    stage[:, m, z0:z0 + zcnt,
                                          y0 // 2:y0 // 2 + R // 2, :],
                                    pw[:, :, 0::2, :], pw[:, :, 1::2, :])
                            ti += 1
                ctx2.__exit__(None, None, None)
                with nc.named_scope(f"ar{name}"):
                    s_t, t_t = bn_reduce(name, stats_t, 128, Mch)

                if name == "5b":
                    # pool5: window (2,2,2) stride 2, pad (0,1,1)
                    pd = small.tile([128, 4, 7, 7], BF16, tag="pd5")
                    nc.vector.tensor_max(pd[:], stage[:, :, 0], stage[:, :, 1])
                    pw5 = small.tile([128, 4, 7, 4], BF16, tag="pw5")
                    nc.vector.tensor_copy(pw5[:, :, :, 0:1], pd[:, :, :, 0:1])
                    nc.vector.tensor_max(pw5[:, :, :, 1:4],
                                         pd[:, :, :, 1::2], pd[:, :, :, 2::2])
                    ph5 = small.tile([128, 4, 4, 4], BF16, tag="ph5")
                    nc.vector.tensor_copy(ph5[:, :, 0:1, :], pw5[:, :, 0:1, :])
                    nc.vector.tensor_max(ph5[:, :, 1:4, :],
                                         pw5[:, :, 1::2, :], pw5[:, :, 2::2, :])
                    # BN+ReLU -> Z, then global mean (1/16 folded into fcw)
                    Z = small.tile([128, 4, 16], BF16, tag="z5")
                    for m in range(4):
                        nc.scalar.activation(
                            Z[:, m, :],
                            ph5[:, m].rearrange("p a b -> p (a b)"),
                            RELU, bias=t_t[:, m:m + 1], scale=s_t[:, m:m + 1])
                    feat = small.tile([128, 4], F32, tag="feat")
                    nc.vector.tensor_reduce(feat[:], Z[:],
                                            axis=mybir.AxisListType.X, op=ADD)
                    fcin = small.tile([128, 4], BF16, tag="fcin")
                    nc.vector.tensor_copy(fcin[:], feat[:])
                    psf = psfc.tile([101, 1], F32, tag="psfc")
                    for c in range(4):
                        nc.tensor.matmul(psf[:], fcw_sb[:, c, :],
                                         fcin[:, c:c + 1],
                                         start=(c == 0), stop=(c == 3))
                    out_sb = small.tile([101, 1], F32, tag="outsb")
                    nc.scalar.activation(out_sb[:], psf[:], IDENT,
                                         bias=fcb_sb[:])
                    nc.sync.dma_start(logits[:], out_sb[:])
                    break

                # D-pool (if pooled) then BN+ReLU apply into next padded input
                if pooled:
                    src = ypoolp.tile([128, Mch, D2, H2, W2], BF16, tag="ypool")
                    for m in range(Mch):
                        nc.vector.tensor_max(src[:, m], stage[:, m, 0::2],
                                             stage[:, m, 1::2])
                    nD, nH, nW = D2, H2, W2
                else:
                    src = stage
                    nD, nH, nW = D, H, W
                Pnext = arena.tile([128, Mch, nD + 2, nH + 2, nW + 2], BF16,
                                   tag="pin")
                zero_borders(Pnext, Mch, nD + 2, nH + 2, nW + 2)
                for m in range(Mch):
                    nc.scalar.activation(
                        Pnext[:, m, 1:1 + nD, 1:1 + nH, 1:1 + nW],
                        src[:, m], RELU,
                        bias=t_t[:, m:m + 1], scale=s_t[:, m:m + 1])
                Pin = Pnext


_STATE = {}


def _get_nc(n_cores=N_CORES):
    key = f"nc{n_cores}"
    if key not in _STATE:
        _STATE[key] = build_bass(n_cores)
    return _STATE[key]


def kernel(**inputs):
    nc = _get_nc()
    shared, x1_list = host_prep(inputs)
    in_maps = []
    for i in range(N_CORES):
        m = dict(shared)
        m["x1"] = x1_list[i]
        in_maps.append(m)
    res = run_bass_kernel_spmd(nc, in_maps, core_ids=list(range(N_CORES)))
    out = np.stack([res.results[i]["logits"].reshape(101)
                    for i in range(N_CORES)]).astype(np.float32)
    return out


# revision 7
# speedup vs baseline: 1.1040x; 1.1040x over previous
"""C3D-style circulant-block 3D CNN forward pass on 8 Trainium2 NeuronCores.

Sharding: data-parallel over batch (8 samples -> 8 cores). Training-mode
BatchNorm batch statistics are combined across cores with a tiny per-layer
f32 AllReduce of (mean, E[x^2]) per channel.

Device kernel per core (per sample):
  conv1 via host-side im2col (K=81 = ci*kd*kh*kw) -> single matmul stream
  conv2..conv5b as shift-and-accumulate implicit GEMM: input channels on
  partitions (K-chunks of 128), 27 taps accumulated in PSUM, strided
  interior access patterns on padded SBUF buffers. conv2 packs (ci, 2 kd
  planes) into K=128 + a K=64 remainder.
  Per conv tile: ACT copies PSUM->bf16, DVE bn_stats accumulates BN stats,
  DVE tensor_max performs maxpool (pool applied to raw values before the
  BN affine; valid because the BN scale g*rsqrt(var+eps) is positive).
  After each conv: bn_aggr -> AllReduce(mean, E2) -> scale/shift -> fused
  BN+ReLU via one ACT op writing the next layer's padded input.
  Tail: special-padded pool5 -> global mean (folded /16 into FC weights)
  -> FC matmul -> logits.
"""

import numpy as np
import ml_dtypes

import concourse.bass as bass
import concourse.mybir as mybir
import concourse.tile as tile
from concourse import bacc
from concourse.bass_utils import run_bass_kernel_spmd

F32 = mybir.dt.float32
BF16 = mybir.dt.bfloat16
NPBF16 = ml_dtypes.bfloat16
RELU = mybir.ActivationFunctionType.Relu
COPY = mybir.ActivationFunctionType.Copy
SQRT = mybir.ActivationFunctionType.Sqrt
IDENT = mybir.ActivationFunctionType.Identity
ADD = mybir.AluOpType.add
EPS = 1e-5
N_CORES = 8

# name, Cin, Cout, D, H, W, R(rows/tile), zpair, pooled
GEN_LAYERS = [
    ("3a", 128, 256, 8, 28, 28, 14, False, False),
    ("3b", 256, 256, 8, 28, 28, 14, False, True),
    ("4a", 256, 512, 4, 14, 14, 14, True, False),
    ("4b", 512, 512, 4, 14, 14, 14, True, True),
    ("5a", 512, 512, 2, 7, 7, 7, True, False),
    ("5b", 512, 512, 2, 7, 7, 7, True, None),  # None -> raw stage (special pool)
]

TAPS = [(kd, kh, kw) for kd in range(3) for kh in range(3) for kw in range(3)]


def circ_expand_np(c):
    c = np.asarray(c, np.float32)
    P, Q, b = c.shape[0], c.shape[1], c.shape[2]
    r = np.arange(b)
    idx = (r[:, None] - r[None, :]) % b
    w = c[:, :, idx]  # (P, Q, b, b, k, k, k)
    w = np.transpose(w, (0, 2, 1, 3, 4, 5, 6))
    return w.reshape(P * b, Q * b, *c.shape[3:])


def pack_w_generic(wd, Kch, Mch):
    # wd (Co, Ci, 3,3,3) -> [Mch, 128ci, Kch, 27, 128co] bf16
    Co, Ci = wd.shape[0], wd.shape[1]
    wt = wd.transpose(1, 2, 3, 4, 0)  # (Ci, kd,kh,kw, Co)
    wt = wt.reshape(Kch, 128, 27, Mch, 128)
    wt = wt.transpose(3, 1, 0, 2, 4)  # (m, ci, c, t, co)
    return np.ascontiguousarray(wt, dtype=NPBF16)


def host_prep(inputs):
    g = {k: np.asarray(v, np.float32) for k, v in inputs.items()}
    shared = {}
    # conv1
    w1 = g["conv1_w"]  # (64, 3, 3,3,3)
    shared["w1"] = np.ascontiguousarray(
        w1.transpose(1, 2, 3, 4, 0).reshape(81, 64), dtype=NPBF16)
    # conv2 (kd-packed)
    w2 = circ_expand_np(g["c2"])  # (128, 64, 3,3,3)
    w2t = w2.transpose(2, 1, 3, 4, 0)  # (kd, ci, kh, kw, co)
    shared["w2a"] = np.ascontiguousarray(
        w2t[0:2].reshape(128, 9, 128), dtype=NPBF16)
    shared["w2b"] = np.ascontiguousarray(
        w2t[2].reshape(64, 9, 128), dtype=NPBF16)
    # generic layers
    for (name, Cin, Cout, *_rest) in GEN_LAYERS:
        wd = circ_expand_np(g[f"c{name}"])
        shared[f"w{name}"] = pack_w_generic(wd, Cin // 128, Cout // 128)
    # bn params
    def pk(v, parts):
        v = np.asarray(v, np.float32)
        mch = v.size // parts
        return np.ascontiguousarray(v.reshape(mch, parts).T)
    shared["gn1"] = pk(g["g1"], 64)
    shared["bn1"] = pk(g["b1"], 64)
    for name, c in [("2", 128), ("3a", 256), ("3b", 256), ("4a", 512),
                    ("4b", 512), ("5a", 512), ("5b", 512)]:
        shared[f"gn{name}"] = pk(g[f"g{name}"], 128)
        shared[f"bn{name}"] = pk(g[f"b{name}"], 128)
        assert np.all(g[f"g{name}"] >= 0), "pool/BN commute needs g >= 0"
    assert np.all(g["g1"] >= 0)
    # fc (fold /16 global-mean into weights)
    fcw = (g["fc_w"].T / 16.0)  # (512, 101)
    shared["fcw"] = np.ascontiguousarray(
        fcw.reshape(4, 128, 101).transpose(1, 0, 2), dtype=NPBF16)
    shared["fcb"] = np.ascontiguousarray(g["fc_b"].reshape(101, 1))
    # per-core conv1 im2col
    x = g["x"]  # (8, 3, 16, 112, 112)
    x1_list = []
    for i in range(x.shape[0]):
        xp = np.zeros((3, 18, 114, 114), np.float32)
        xp[:, 1:17, 1:113, 1:113] = x[i]
        sw = np.lib.stride_tricks.sliding_window_view(xp, (3, 3, 3), axis=(1, 2, 3))
        b1 = sw.transpose(0, 4, 5, 6, 1, 2, 3).reshape(81, 16, 12544)
        x1_list.append(np.ascontiguousarray(b1, dtype=NPBF16))
    return shared, x1_list


def build_bass(n_cores, fake_cc=False):
    nc = bacc.Bacc("TRN2", target_bir_lowering=False, debug=False,
                   num_devices=n_cores)
    rg = [list(range(n_cores))]

    din = {}
    din["x1"] = nc.dram_tensor("x1", [81, 16, 12544], BF16, kind="ExternalInput")
    din["w1"] = nc.dram_tensor("w1", [81, 64], BF16, kind="ExternalInput")
    din["w2a"] = nc.dram_tensor("w2a", [128, 9, 128], BF16, kind="ExternalInput")
    din["w2b"] = nc.dram_tensor("w2b", [64, 9, 128], BF16, kind="ExternalInput")
    for (name, Cin, Cout, *_r) in GEN_LAYERS:
        din[f"w{name}"] = nc.dram_tensor(
            f"w{name}", [Cout // 128, 128, Cin // 128, 27, 128], BF16,
            kind="ExternalInput")
    din["gn1"] = nc.dram_tensor("gn1", [64, 1], F32, kind="ExternalInput")
    din["bn1"] = nc.dram_tensor("bn1", [64, 1], F32, kind="ExternalInput")
    for name, c in [("2", 128), ("3a", 256), ("3b", 256), ("4a", 512),
                    ("4b", 512), ("5a", 512), ("5b", 512)]:
        mch = c // 128
        din[f"gn{name}"] = nc.dram_tensor(f"gn{name}", [128, mch], F32,
                                          kind="ExternalInput")
        din[f"bn{name}"] = nc.dram_tensor(f"bn{name}", [128, mch], F32,
                                          kind="ExternalInput")
    din["fcw"] = nc.dram_tensor("fcw", [128, 4, 101], BF16, kind="ExternalInput")
    din["fcb"] = nc.dram_tensor("fcb", [101, 1], F32, kind="ExternalInput")
    logits = nc.dram_tensor("logits", [101, 1], F32, kind="ExternalOutput")
    stats_out = {}
    for name, parts, mch in [("1", 64, 1), ("2", 128, 1), ("3a", 128, 2),
                             ("3b", 128, 2), ("4a", 128, 4), ("4b", 128, 4),
                             ("5a", 128, 4), ("5b", 128, 4)]:
        stats_out[name] = nc.dram_tensor(f"st{name}", [parts, mch * 2], F32,
                                         kind="ExternalOutput")

    with tile.TileContext(nc) as tc:
        build_graph(tc, din, logits, stats_out, rg, fake_cc)
    nc.compile()
    return nc


def build_graph(tc, din, logits, stats_out, rg, fake_cc=False):
    nc = tc.nc
    import contextlib
    ctx = contextlib.ExitStack()
    with ctx:
        singles = ctx.enter_context(tc.tile_pool(name="singles", bufs=1))
        small = ctx.enter_context(tc.tile_pool(name="small", bufs=2))
        statsp = ctx.enter_context(tc.tile_pool(name="statsp", bufs=2))
        psum = ctx.enter_context(tc.tile_pool(name="psum", bufs=3, space="PSUM"))
        psfc = ctx.enter_context(tc.tile_pool(name="psfc", bufs=1, space="PSUM"))
        ybfp = ctx.enter_context(tc.tile_pool(name="ybfp", bufs=4))
        pwp = ctx.enter_context(tc.tile_pool(name="pwp", bufs=4))
        stagep = ctx.enter_context(tc.tile_pool(name="stagep", bufs=1))
        ypoolp = ctx.enter_context(tc.tile_pool(name="ypoolp", bufs=1))
        arena = ctx.enter_context(tc.tile_pool(name="arena", bufs=1))
        dram = ctx.enter_context(tc.tile_pool(name="dram", bufs=1, space="DRAM"))

        eps_t = singles.tile([128, 1], F32, tag="eps")
        nc.vector.memset(eps_t[:], EPS)

        # persistent small params
        params = {}
        for name, parts in [("1", 64), ("2", 128), ("3a", 128), ("3b", 128),
                            ("4a", 128), ("4b", 128), ("5a", 128), ("5b", 128)]:
            mch = din[f"gn{name}"].shape[1]
            gt = singles.tile([parts, mch], F32, tag=f"g{name}")
            bt = singles.tile([parts, mch], F32, tag=f"b{name}")
            nc.sync.dma_start(gt[:], din[f"gn{name}"][:])
            nc.sync.dma_start(bt[:], din[f"bn{name}"][:])
            params[name] = (gt, bt)

        w1_sb = singles.tile([81, 64], BF16, tag="w1")
        nc.sync.dma_start(w1_sb[:], din["w1"][:])
        w2a_sb = singles.tile([128, 9, 128], BF16, tag="w2a")
        nc.sync.dma_start(w2a_sb[:], din["w2a"][:])
        w2b_sb = singles.tile([64, 9, 128], BF16, tag="w2b")
        nc.sync.dma_start(w2b_sb[:], din["w2b"][:])
        fcw_sb = singles.tile([128, 4, 101], BF16, tag="fcw")
        nc.sync.dma_start(fcw_sb[:], din["fcw"][:])
        fcb_sb = singles.tile([101, 1], F32, tag="fcb")
        nc.sync.dma_start(fcb_sb[:], din["fcb"][:])

        y1_dram = dram.tile([64, 16, 3136], BF16, tag="y1d")

        def bn_reduce(name, stats_t, parts, mch):
            """stats_t [parts, mch, T, 6] -> (s, t) [parts, mch] f32."""
            mv = small.tile([parts, mch, 2], F32, tag="mv")
            for m in range(mch):
                nc.vector.bn_aggr(mv[:, m], stats_t[:, m])
            cc = small.tile([parts, mch, 2], F32, tag="cc")
            sq = small.tile([parts, mch], F32, tag="sq")
            nc.vector.tensor_mul(sq[:], mv[:, :, 0], mv[:, :, 0])
            nc.vector.tensor_add(cc[:, :, 1], mv[:, :, 1], sq[:])
            nc.vector.tensor_copy(cc[:, :, 0], mv[:, :, 0])
            ccin = dram.tile([parts, mch * 2], F32, tag=f"ccin{name}")
            ccout = dram.tile([parts, mch * 2], F32, tag=f"ccout{name}",
                              addr_space="Shared")
            nc.sync.dma_start(ccin[:], cc[:].rearrange("p m two -> p (m two)"))
            if fake_cc:
                nc.sync.dma_start(ccout[:], ccin[:])
            else:
                nc.gpsimd.collective_compute(
                    "AllReduce", ADD, replica_groups=rg,
                    ins=[ccin.opt()], outs=[ccout.opt()])
            ar = small.tile([parts, mch, 2], F32, tag="ar")
            nc.sync.dma_start(ar[:].rearrange("p m two -> p (m two)"), ccout[:])
            nc.sync.dma_start(stats_out[name][:],
                              ar[:].rearrange("p m two -> p (m two)"))
            inv_n = 1.0 / len(rg[0])
            mg = small.tile([parts, mch], F32, tag="mg")
            e2 = small.tile([parts, mch], F32, tag="e2")
            nc.vector.tensor_scalar_mul(mg[:], ar[:, :, 0], inv_n)
            nc.vector.tensor_scalar_mul(e2[:], ar[:, :, 1], inv_n)
            sq2 = small.tile([parts, mch], F32, tag="sq2")
            nc.vector.tensor_mul(sq2[:], mg[:], mg[:])
            varg = small.tile([parts, mch], F32, tag="varg")
            nc.vector.tensor_sub(varg[:], e2[:], sq2[:])
            sd = small.tile([parts, mch], F32, tag="sd")
            nc.scalar.activation(sd[:], varg[:], SQRT, bias=eps_t[:parts])
            inv = small.tile([parts, mch], F32, tag="inv")
            nc.vector.reciprocal(inv[:], sd[:])
            gt, bt = params[name]
            s_t = small.tile([parts, mch], F32, tag="s_t")
            nc.vector.tensor_mul(s_t[:], inv[:], gt[:])
            tmn = small.tile([parts, mch], F32, tag="tmn")
            nc.vector.tensor_mul(tmn[:], mg[:], s_t[:])
            t_t = small.tile([parts, mch], F32, tag="t_t")
            nc.vector.tensor_sub(t_t[:], bt[:], tmn[:])
            return s_t, t_t

        def zero_borders(P, mch, Dp, Hp, Wp):
            for c in range(mch):
                nc.vector.memset(P[:, c, 0], 0.0)
                nc.vector.memset(P[:, c, Dp - 1], 0.0)
                nc.vector.memset(P[:, c, 1:Dp - 1, 0, :], 0.0)
                nc.vector.memset(P[:, c, 1:Dp - 1, Hp - 1, :], 0.0)
                nc.vector.memset(P[:, c, 1:Dp - 1, 1:Hp - 1, 0:1], 0.0)
                nc.vector.memset(P[:, c, 1:Dp - 1, 1:Hp - 1, Wp - 1:Wp], 0.0)

        # ---------------- conv1 ----------------
        stats1 = statsp.tile([64, 1, 448, 6], F32, tag="stats")
        with tc.tile_pool(name="x1p", bufs=3) as x1p, \
             tc.tile_pool(name="zplp", bufs=2) as zplp, \
             nc.named_scope("conv1"):
            for z in range(16):
                zplane = zplp.tile([64, 56, 56], BF16, tag="zpl")
                for half in range(2):
                    slab = x1p.tile([81, 6272], BF16, tag="slab")
                    nc.sync.dma_start(
                        slab[:], din["x1"][:, z, half * 6272:(half + 1) * 6272])
                    for t2 in range(7):
                        pst = psum.tile([64, 2, 512], F32, tag="ps", name="ps")
                        for j in range(2):
                            t = 2 * t2 + j
                            nc.tensor.matmul(pst[:, j, :448], w1_sb[:],
                                             slab[:, t * 448:(t + 1) * 448],
                                             start=True, stop=True)
                        ybft = ybfp.tile([64, 1024], BF16, tag="ybf",
                                         name="ybf")
                        ybf = ybft[:, :896]
                        nc.scalar.activation(
                            ybf.rearrange("p (g n) -> p g n", g=2),
                            pst[:, :, :448], COPY)
                        ti = z * 28 + half * 14 + 2 * t2
                        nc.vector.bn_stats(stats1[:, 0, ti], ybft[:, 0:448])
                        nc.vector.bn_stats(stats1[:, 0, ti + 1],
                                           ybft[:, 448:896])
                        v = ybf.rearrange("p (a b) -> p a b", a=8)
                        pw = pwp.tile([64, 8, 56], BF16, tag="pw")
                        nc.vector.tensor_max(pw[:], v[:, :, 0::2], v[:, :, 1::2])
                        ro = half * 28 + 4 * t2
                        nc.vector.tensor_max(zplane[:, ro:ro + 4, :],
                                             pw[:, 0::2, :], pw[:, 1::2, :])
                nc.sync.dma_start(y1_dram[:, z, :],
                                  zplane[:].rearrange("p a b -> p (a b)"))
        with nc.named_scope("ar1"):
            s1, t1 = bn_reduce("1", stats1, 64, 1)

        # ---------------- conv2 ----------------
        stats2 = statsp.tile([128, 1, 128, 6], F32, tag="stats")
        S2 = stagep.tile([128, 1, 16, 28, 28], BF16, tag="stage")
        with tc.tile_pool(name="plp", bufs=3) as plp, \
             tc.tile_pool(name="b2p", bufs=2) as b2p, \
             tc.tile_pool(name="c2p", bufs=2) as c2p, \
             nc.named_scope("conv2"):

            def build_plane(dst64, pidx):
                if pidx == 0 or pidx == 17:
                    nc.vector.memset(dst64[:], 0.0)
                    return
                pl = plp.tile([64, 3136], BF16, tag="pl")
                nc.sync.dma_start(pl[:], y1_dram[:, pidx - 1, :])
                nc.vector.memset(dst64[:, 0, :], 0.0)
                nc.vector.memset(dst64[:, 57, :], 0.0)
                nc.vector.memset(dst64[:, 1:57, 0:1], 0.0)
                nc.vector.memset(dst64[:, 1:57, 57:58], 0.0)
                nc.scalar.activation(
                    dst64[:, 1:57, 1:57],
                    pl[:].rearrange("p (a b) -> p a b", a=56),
                    RELU, bias=t1[:, 0:1], scale=s1[:, 0:1])

            for z in range(16):
                B2 = b2p.tile([128, 58, 58], BF16, tag="b2")
                build_plane(B2[0:64], z)
                build_plane(B2[64:128], z + 1)
                C2 = c2p.tile([64, 58, 58], BF16, tag="c2")
                build_plane(C2[:], z + 2)
                for p2 in range(4):
                    pst = psum.tile([128, 2, 512], F32, tag="ps", name="ps")
                    for k9 in range(9):
                        kh, kw = k9 // 3, k9 % 3
                        for j in range(2):
                            y0 = 14 * p2 + 7 * j + kh
                            nc.tensor.matmul(pst[:, j, :392], w2a_sb[:, k9, :],
                                             B2[:, y0:y0 + 7, kw:kw + 56],
                                             start=(k9 == 0), stop=False)
                    for k9 in range(9):
                        kh, kw = k9 // 3, k9 % 3
                        for j in range(2):
                            y0 = 14 * p2 + 7 * j + kh
                            nc.tensor.matmul(pst[:, j, :392], w2b_sb[:, k9, :],
                                             C2[:, y0:y0 + 7, kw:kw + 56],
                                             start=False, stop=(k9 == 8))
                    ybft = ybfp.tile([128, 1024], BF16, tag="ybf", name="ybf")
                    ybf = ybft[:, :784]
                    nc.vector.tensor_copy(
                        ybf.rearrange("p (g n) -> p g n", g=2),
                        pst[:, :, :392])
                    nc.vector.bn_stats(stats2[:, 0, z * 8 + 2 * p2],
                                       ybft[:, 0:392])
                    nc.vector.bn_stats(stats2[:, 0, z * 8 + 2 * p2 + 1],
                                       ybft[:, 392:784])
                    v = ybf.rearrange("p (a b) -> p a b", a=14)
                    pw = pwp.tile([128, 14, 28], BF16, tag="pw")
                    nc.vector.tensor_max(pw[:], v[:, :, 0::2], v[:, :, 1::2])
                    nc.vector.tensor_max(S2[:, 0, z, 7 * p2:7 * p2 + 7, :],
                                         pw[:, 0::2, :], pw[:, 1::2, :])
        with nc.named_scope("ar2"):
            s2, t2 = bn_reduce("2", stats2, 128, 1)
        Y2p = ypoolp.tile([128, 1, 8, 28, 28], BF16, tag="ypool")
        nc.vector.tensor_max(Y2p[:, 0], S2[:, 0, 0::2], S2[:, 0, 1::2])
        P3in = arena.tile([128, 1, 10, 30, 30], BF16, tag="pin")
        zero_borders(P3in, 1, 10, 30, 30)
        nc.scalar.activation(P3in[:, 0, 1:9, 1:29, 1:29], Y2p[:, 0], RELU,
                             bias=t2[:, 0:1], scale=s2[:, 0:1])

        # ---------------- generic conv layers ----------------
        with tc.tile_pool(name="wp", bufs=3) as wp:
            Pin = P3in
            for (name, Cin, Cout, D, H, W, R, zpair, pooled) in GEN_LAYERS:
                Kch, Mch = Cin // 128, Cout // 128
                ntz = D // 2 if zpair else D
                zcnt = 2 if zpair else 1
                ytiles = H // R
                N = zcnt * R * W
                T = ntz * ytiles
                H2, W2, D2 = H // 2, W // 2, D // 2
                stats_t = statsp.tile([128, Mch, T, 6], F32, tag="stats")
                if pooled is False or pooled is None:
                    stage = stagep.tile([128, Mch, D, H, W], BF16, tag="stage")
                else:
                    stage = stagep.tile([128, Mch, D, H2, W2], BF16, tag="stage")
                stage_flat = stage[:].rearrange("p m d h w -> p (m d h w)")
                scope = ctx2 = nc.named_scope(f"conv{name}")
                ctx2.__enter__()
                tiles = [(2 * tz if zpair else tz, ty * R)
                         for tz in range(ntz) for ty in range(ytiles)]
                groups = [tiles[i:i + 2] for i in range(0, len(tiles), 2)]
                for m in range(Mch):
                    wm = wp.tile([128, Kch, 27, 128], BF16, tag="w")
                    nc.sync.dma_start(wm[:, :Kch], din[f"w{name}"][m])
                    ti = 0
                    for grp in groups:
                        G = len(grp)
                        pst = psum.tile([128, 2, 512], F32, tag="ps",
                                        name="ps")
                        nmm = Kch * 27
                        i = 0
                        for c in range(Kch):
                            for (kd, kh, kw) in TAPS:
                                for j, (z0, y0) in enumerate(grp):
                                    rhs = Pin[:, c, z0 + kd:z0 + kd + zcnt,
                                              y0 + kh:y0 + kh + R,
                                              kw:kw + W]
                                    nc.tensor.matmul(
                                        pst[:, j, :N],
                                        wm[:, c, kd * 9 + kh * 3 + kw, :],
                                        rhs, start=(i == 0), stop=(i == nmm - 1))
                                i += 1
                        z0, y0 = grp[0]
                        if pooled is False or pooled is None:
                            off = (m * D + z0) * H * W + y0 * W
                            dst = stage_flat[:, off:off + G * N]
                            nc.vector.tensor_copy(
                                dst.rearrange("p (g n) -> p g n", g=G),
                                pst[:, :G, :N])
                            for j in range(G):
                                nc.vector.bn_stats(
                                    stats_t[:, m, ti + j],
                                    stage_flat[:, off + j * N:off + (j + 1) * N])
                        else:
                            ybft = ybfp.tile([128, 1024], BF16,
                                             tag="ybf", name="ybf")
                            ybf = ybft[:, :G * N]
                            nc.vector.tensor_copy(
                                ybf.rearrange("p (g n) -> p g n", g=G),
                                pst[:, :G, :N])
                            for j in range(G):
                                nc.vector.bn_stats(
                                    stats_t[:, m, ti + j],
                                    ybft[:, j * N:(j + 1) * N])
                            nz = G * zcnt if zpair else 1
                            nr = R if zpair else G * R
                            v = ybf.rearrange("p (z y x) -> p z y x",
                                              z=nz, y=nr)
                            pw = pwp.tile([128, nz, nr, W2], BF16,
                                          tag="pw2", name="pw")
                            nc.vector.tensor_max(pw[:], v[:, :, :, 0::2],
                                                 v[:, :, :, 1::2])
                            nc.vector.tensor_max(
                                stage[:, m, z0:z0 + nz,
                                      y0 // 2:y0 // 2 + nr // 2, :],
                                pw[:, :, 0::2, :], pw[:, :, 1::2, :])
                        ti += G
                ctx2.__exit__(None, None, None)
                with nc.named_scope(f"ar{name}"):
                    s_t, t_t = bn_reduce(name, stats_t, 128, Mch)

                if name == "5b":
                    # pool5: window (2,2,2) stride 2, pad (0,1,1)
                    pd = small.tile([128, 4, 7, 7], BF16, tag="pd5")
                    nc.vector.tensor_max(pd[:], stage[:, :, 0], stage[:, :, 1])
                    pw5 = small.tile([128, 4, 7, 4], BF16, tag="pw5")
                    nc.vector.tensor_copy(pw5[:, :, :, 0:1], pd[:, :, :, 0:1])
                    nc.vector.tensor_max(pw5[:, :, :, 1:4],
                                         pd[:, :, :, 1::2], pd[:, :, :, 2::2])
                    ph5 = small.tile([128, 4, 4, 4], BF16, tag="ph5")
                    nc.vector.tensor_copy(ph5[:, :, 0:1, :], pw5[:, :, 0:1, :])
                    nc.vector.tensor_max(ph5[:, :, 1:4, :],
                                         pw5[:, :, 1::2, :], pw5[:, :, 2::2, :])
                    # BN+ReLU -> Z, then global mean (1/16 folded into fcw)
                    Z = small.tile([128, 4, 16], BF16, tag="z5")
                    for m in range(4):
                        nc.scalar.activation(
                            Z[:, m, :],
                            ph5[:, m].rearrange("p a b -> p (a b)"),
                            RELU, bias=t_t[:, m:m + 1], scale=s_t[:, m:m + 1])
                    feat = small.tile([128, 4], F32, tag="feat")
                    nc.vector.tensor_reduce(feat[:], Z[:],
                                            axis=mybir.AxisListType.X, op=ADD)
                    fcin = small.tile([128, 4], BF16, tag="fcin")
                    nc.vector.tensor_copy(fcin[:], feat[:])
                    psf = psfc.tile([101, 1], F32, tag="psfc")
                    for c in range(4):
                        nc.tensor.matmul(psf[:], fcw_sb[:, c, :],
                                         fcin[:, c:c + 1],
                                         start=(c == 0), stop=(c == 3))
                    out_sb = small.tile([101, 1], F32, tag="outsb")
                    nc.scalar.activation(out_sb[:], psf[:], IDENT,
                                         bias=fcb_sb[:])
                    nc.sync.dma_start(logits[:], out_sb[:])
                    break

                # D-pool (if pooled) then BN+ReLU apply into next padded input
                if pooled:
                    src = ypoolp.tile([128, Mch, D2, H2, W2], BF16, tag="ypool")
                    for m in range(Mch):
                        nc.vector.tensor_max(src[:, m], stage[:, m, 0::2],
                                             stage[:, m, 1::2])
                    nD, nH, nW = D2, H2, W2
                else:
                    src = stage
                    nD, nH, nW = D, H, W
                Pnext = arena.tile([128, Mch, nD + 2, nH + 2, nW + 2], BF16,
                                   tag="pin")
                zero_borders(Pnext, Mch, nD + 2, nH + 2, nW + 2)
                for m in range(Mch):
                    nc.scalar.activation(
                        Pnext[:, m, 1:1 + nD, 1:1 + nH, 1:1 + nW],
                        src[:, m], RELU,
                        bias=t_t[:, m:m + 1], scale=s_t[:, m:m + 1])
                Pin = Pnext


_STATE = {}


def _get_nc(n_cores=N_CORES):
    key = f"nc{n_cores}"
    if key not in _STATE:
        _STATE[key] = build_bass(n_cores)
    return _STATE[key]


def kernel(**inputs):
    nc = _get_nc()
    shared, x1_list = host_prep(inputs)
    in_maps = []
    for i in range(N_CORES):
        m = dict(shared)
        m["x1"] = x1_list[i]
        in_maps.append(m)
    res = run_bass_kernel_spmd(nc, in_maps, core_ids=list(range(N_CORES)))
    out = np.stack([res.results[i]["logits"].reshape(101)
                    for i in range(N_CORES)]).astype(np.float32)
    return out
